# revision 9
# baseline (speedup 1.0000x reference)
"""Trainium2 Bass kernel for nn_Pndb_43344809951805 (scatter_memory).

Data-parallel over batch B=16 across 8 NeuronCores (2 batches/core).
Phase 1 writes the [Q,D] memory A (mean over B -> AllReduce), phase 2 reads it.
All big matmuls run in bf16 (full PE rate); residual path stays f32.
"""
import sys

sys.path.insert(0, "/opt/trn_rl_repo")

import numpy as np
import ml_dtypes

import concourse.bass as bass
import concourse.bacc as bacc
import concourse.mybir as mybir
import concourse.tile as tile
from concourse import masks
from concourse.bass_utils import run_bass_kernel_spmd

F32 = mybir.dt.float32
BF16 = mybir.dt.bfloat16
AF = mybir.ActivationFunctionType
ALU = mybir.AluOpType
BF = ml_dtypes.bfloat16

B, S, D, Q = 16, 2048, 1024, 64
NCORES = 8
BL = B // NCORES          # local batches per core
SBLK = 512                # s-block (matmul moving free dim)
NSB = S // SBLK           # 4 s-blocks per batch
NCH = S // 128            # 16 s-chunks per batch
NJ = D // 128             # 8 contraction chunks
NI = D // 128             # 8 output-dim chunks
CPB = SBLK // 128         # 4 chunks per s-block

_prog_cache = {}


def _build(bi_v: float, cgate_v: float, stage: str = "full"):
    nc = bacc.Bacc("TRN2", target_bir_lowering=False, debug=False,
                   enable_asserts=False, num_devices=NCORES)

    rawT_d = nc.dram_tensor("rawT", [BL, D, S], BF16, kind="ExternalInput")
    rawN_d = nc.dram_tensor("rawN", [BL, S, D], BF16, kind="ExternalInput")
    pdT_d = nc.dram_tensor("pdT", [BL, D, S], BF16, kind="ExternalInput")
    pdN_d = nc.dram_tensor("pdN", [BL, S, D], F32, kind="ExternalInput")
    wkT_d = nc.dram_tensor("wkT", [D, D], BF16, kind="ExternalInput")
    woT_d = nc.dram_tensor("woT", [D, D], BF16, kind="ExternalInput")
    qT1_d = nc.dram_tensor("qT1", [D, Q], BF16, kind="ExternalInput")
    qT2_d = nc.dram_tensor("qT2", [D, Q], BF16, kind="ExternalInput")
    bkT_d = nc.dram_tensor("bkT", [D, 1], F32, kind="ExternalInput")
    boT_d = nc.dram_tensor("boT", [D, 1], F32, kind="ExternalInput")
    wiB_d = nc.dram_tensor("wiB", [128, D], BF16, kind="ExternalInput")
    wu1B_d = nc.dram_tensor("wu1B", [128, D], F32, kind="ExternalInput")
    wu2B_d = nc.dram_tensor("wu2B", [Q, D], F32, kind="ExternalInput")
    out_d = nc.dram_tensor("out", [BL, S, D], F32, kind="ExternalOutput")

    with tile.TileContext(nc) as tc:
        with (
            tc.tile_pool(name="const", bufs=1) as cp,
            tc.tile_pool(name="dram", bufs=1, space="DRAM") as dram,
        ):
            # ---- phase-1-critical constants first (DMA order matters) ----
            ident = cp.tile([128, 128], BF16, tag="ident")
            masks.make_identity(nc, ident[:])
            ones1 = cp.tile([1, 128], BF16, tag="ones1")
            nc.vector.memset(ones1[:], 1.0)
            nbi = cp.tile([128, 1], F32, tag="nbi")
            nc.vector.memset(nbi[:], -bi_v)
            ncg = cp.tile([128, 1], F32, tag="ncg")
            nc.vector.memset(ncg[:], -cgate_v)

            wkT = [cp.tile([128, D], BF16, tag=f"wkT{j}", name=f"wkT{j}")
                   for j in range(NJ)]
            woT = [cp.tile([128, D], BF16, tag=f"woT{j}", name=f"woT{j}")
                   for j in range(NJ)]
            qT1 = [cp.tile([128, Q], BF16, tag=f"qT1{j}", name=f"qT1{j}")
                   for j in range(NJ)]
            qT2 = [cp.tile([128, Q], BF16, tag=f"qT2{j}", name=f"qT2{j}")
                   for j in range(NJ)]
            bkT = [cp.tile([128, 1], F32, tag=f"bkT{j}", name=f"bkT{j}")
                   for j in range(NJ)]
            boT = [cp.tile([128, 1], F32, tag=f"boT{j}", name=f"boT{j}")
                   for j in range(NJ)]
            wiB = cp.tile([128, D], BF16, tag="wiB")
            wu1B = cp.tile([128, D], F32, tag="wu1B")
            wu2B = cp.tile([Q, D], F32, tag="wu2B")
            for j in range(NJ):
                sl = slice(j * 128, (j + 1) * 128)
                nc.sync.dma_start(wkT[j][:], wkT_d[sl, :])
                nc.sync.dma_start(qT1[j][:], qT1_d[sl, :])
                nc.sync.dma_start(bkT[j][:], bkT_d[sl, :])
            nc.sync.dma_start(wiB[:], wiB_d[:])

            A_acc = cp.tile([Q, D], F32, tag="A_acc")
            A_f32 = cp.tile([Q, D], F32, tag="A_f32")
            A_bf = cp.tile([Q, D], BF16, tag="A_bf")
            awB = cp.tile([128, Q], BF16, tag="awB")
            scrA = cp.tile([Q, D], F32, tag="scrA")
            aw = cp.tile([Q, 1], F32, tag="aw")
            awb16 = cp.tile([Q, 1], BF16, tag="awb16")
            aw_row = cp.tile([1, Q], BF16, tag="aw_row")

            ar_in = dram.tile([Q, D], F32)
            ar_out = dram.tile([Q, D], F32)

            # ================= PHASE 1 =================
            with (
                tc.tile_pool(name="p1", bufs=1) as p1,
                tc.tile_pool(name="p1ps", bufs=1, space="PSUM") as p1ps,
            ):
                def load_rawt(b, sb):
                    ts = []
                    for j in range(NJ):
                        t = p1.tile([128, SBLK], BF16, tag=f"rawt{j}",
                                    name=f"rawt{j}", bufs=2)
                        nc.sync.dma_start(
                            t[:], rawT_d[b, j * 128:(j + 1) * 128,
                                         sb * SBLK:(sb + 1) * SBLK])
                        ts.append(t)
                    return ts

                for b in range(BL):
                    rawt = load_rawt(b, 0)
                    U = p1.tile([Q, S], BF16, tag="U", bufs=2)
                    Zp = p1.tile([Q, NSB], F32, tag="Zp", bufs=2)
                    A_ps = p1ps.tile([Q, D], F32, tag="A_ps", bufs=1)

                    for sb in range(NSB):
                        # v-gate pre-pass for this s-block's chunks
                        Gg = p1.tile([128, CPB], F32, tag="Gg", bufs=2)
                        rns = []
                        for cc in range(CPB):
                            c = sb * CPB + cc
                            rn = p1.tile([128, D], BF16, tag=f"rawn{cc}",
                                         name=f"rawn{cc}", bufs=2)
                            nc.sync.dma_start(
                                rn[:], rawN_d[b, c * 128:(c + 1) * 128, :])
                            rns.append(rn)
                            scr = p1.tile([128, D], BF16, tag="scrb", bufs=2)
                            nc.vector.scalar_tensor_tensor(
                                scr[:], rn[:], 1.0, wiB[:],
                                ALU.mult, ALU.mult,
                                accum_out=Gg[:, cc:cc + 1])
                        nc.scalar.activation(Gg[:], Gg[:], AF.Exp,
                                             scale=-1.0, bias=nbi[:])
                        nc.vector.tensor_scalar_add(Gg[:], Gg[:], 1.0)
                        nc.vector.reciprocal(Gg[:], Gg[:])

                        nxt = load_rawt(b, sb + 1) if sb + 1 < NSB else None
                        if b == 0 and sb == 0:
                            # phase-2 weights: off the critical DMA path
                            for j in range(NJ):
                                sl = slice(j * 128, (j + 1) * 128)
                                nc.sync.dma_start(woT[j][:], woT_d[sl, :])
                                nc.sync.dma_start(qT2[j][:], qT2_d[sl, :])
                                nc.sync.dma_start(boT[j][:], boT_d[sl, :])
                            nc.sync.dma_start(wu1B[:], wu1B_d[:])
                            nc.sync.dma_start(wu2B[:], wu2B_d[:])

                        # kT matmuls + scores (software-pipelined by one i)
                        sc_ps = p1ps.tile([Q, SBLK], F32, tag="sc_ps", bufs=2)
                        kts = []
                        for i in range(NI):
                            isl = slice(i * 128, (i + 1) * 128)
                            k_ps = p1ps.tile([128, SBLK], F32, tag="k_ps",
                                             bufs=2)
                            for j in range(NJ):
                                nc.tensor.matmul(
                                    k_ps[:], wkT[j][:, isl], rawt[j][:],
                                    start=(j == 0), stop=(j == NJ - 1))
                            kt = p1.tile([128, SBLK], BF16, tag="kt", bufs=3)
                            nc.scalar.activation(kt[:], k_ps[:], AF.Identity,
                                                 bias=bkT[i][:])
                            kts.append(kt)
                            if i >= 1:
                                nc.tensor.matmul(
                                    sc_ps[:], qT1[i - 1][:], kts[i - 1][:],
                                    start=(i - 1 == 0), stop=False,
                                    skip_group_check=True)
                        nc.tensor.matmul(
                            sc_ps[:], qT1[NI - 1][:], kts[NI - 1][:],
                            start=False, stop=True, skip_group_check=True)

                        ssl = slice(sb * SBLK, (sb + 1) * SBLK)
                        nc.scalar.activation(U[:, ssl], sc_ps[:], AF.Exp,
                                             accum_out=Zp[:, sb:sb + 1])
                        # transposes first (decoupled from A matmuls)
                        uts = []
                        for cc in range(CPB):
                            c = sb * CPB + cc
                            ut_ps = p1ps.tile([128, Q], BF16, tag="ut_ps",
                                              bufs=2)
                            nc.tensor.transpose(
                                ut_ps[:], U[:, c * 128:(c + 1) * 128],
                                ident[:Q, :Q])
                            ut = p1.tile([128, Q], BF16, tag="ut", bufs=6)
                            nc.scalar.copy(ut[:], ut_ps[:])
                            uts.append(ut)
                        for cc in range(CPB):
                            c = sb * CPB + cc
                            v = p1.tile([128, D], BF16, tag="v", bufs=3)
                            nc.vector.tensor_scalar_mul(
                                v[:], rns[cc][:], Gg[:, cc:cc + 1])
                            for h in range(2):
                                hsl = slice(h * 512, (h + 1) * 512)
                                nc.tensor.matmul(
                                    A_ps[:, hsl], uts[cc][:], v[:, hsl],
                                    start=(c == 0), stop=(c == NCH - 1),
                                    skip_group_check=True)
                        rawt = nxt

                    # A_acc += A_ps / (16 * Z)
                    Z1 = p1.tile([Q, 1], F32, tag="Z1", bufs=2)
                    nc.vector.tensor_reduce(Z1[:], Zp[:], mybir.AxisListType.X,
                                            ALU.add)
                    sA = p1.tile([Q, 1], F32, tag="sA", bufs=2)
                    nc.vector.reciprocal(sA[:], Z1[:])
                    nc.vector.tensor_scalar_mul(sA[:], sA[:], 1.0 / B)
                    if b == 0:
                        nc.vector.tensor_scalar_mul(A_acc[:], A_ps[:], sA[:])
                    else:
                        nc.vector.scalar_tensor_tensor(
                            A_acc[:], A_ps[:], sA[:], A_acc[:],
                            ALU.mult, ALU.add)

                if stage != "p2":
                    nc.gpsimd.dma_start(ar_in[:], A_acc[:])

            # ---- AllReduce of partial A across the 8 cores ----
            if stage == "p1":
                nc.sync.dma_start(out_d[0, 0:Q, :], A_acc[:])
            elif stage == "p2":
                nc.vector.tensor_copy(A_f32[:], A_acc[:])
            else:
                nc.gpsimd.collective_compute(
                    "AllReduce", ALU.add,
                    replica_groups=[list(range(NCORES))],
                    ins=[ar_in.opt()], outs=[ar_out.opt()],
                )
                nc.sync.dma_start(A_f32[:], ar_out[:])

            # ================= PHASE 2 =================
            if stage == "p1":
                pass  # skip phase 2
            else:
              with (
                  tc.tile_pool(name="p2", bufs=1) as p2,
                  tc.tile_pool(name="p2ps", bufs=1, space="PSUM") as p2ps,
              ):
                  def load_pdt(b, sb):
                      ts = []
                      for j in range(NJ):
                          t = p2.tile([128, SBLK], BF16, tag=f"pdt{j}",
                                      name=f"pdt{j}", bufs=2)
                          nc.sync.dma_start(
                              t[:], pdT_d[b, j * 128:(j + 1) * 128,
                                          sb * SBLK:(sb + 1) * SBLK])
                          ts.append(t)
                      return ts

                  for b in range(BL):
                      pdt = load_pdt(b, 0)
                      for sb in range(NSB):
                          nxt = load_pdt(b, sb + 1) if sb + 1 < NSB else None
                          kot = []
                          for i in range(NI):
                              isl = slice(i * 128, (i + 1) * 128)
                              ko_ps = p2ps.tile([128, SBLK], F32, tag="ko_ps",
                                                bufs=2)
                              for j in range(NJ):
                                  nc.tensor.matmul(
                                      ko_ps[:], woT[j][:, isl], pdt[j][:],
                                      start=(j == 0), stop=(j == NJ - 1))
                              kt = p2.tile([128, SBLK], BF16, tag=f"kot{i}",
                                           name=f"kot{i}", bufs=2)
                              nc.scalar.activation(kt[:], ko_ps[:],
                                                   AF.Identity,
                                                   bias=boT[i][:])
                              kot.append(kt)
                          # part A: s2 + exp + g1 per chunk
                          G1 = p2.tile([128, CPB], F32, tag="G1", bufs=2)
                          G2 = p2.tile([128, CPB], F32, tag="G2", bufs=2)
                          Z2 = p2.tile([128, CPB], F32, tag="Z2", bufs=2)
                          pdn = []
                          u2s = []
                          for cc in range(CPB):
                              c = sb * CPB + cc
                              pn = p2.tile([128, D], F32, tag=f"pdn{cc}",
                                           name=f"pdn{cc}", bufs=2)
                              nc.sync.dma_start(
                                  pn[:], pdN_d[b, c * 128:(c + 1) * 128, :])
                              pdn.append(pn)
                              scr = p2.tile([128, D], F32, tag="scrf", bufs=2)
                              nc.vector.scalar_tensor_tensor(
                                  scr[:], pn[:], 1.0, wu1B[:],
                                  ALU.mult, ALU.mult,
                                  accum_out=G1[:, cc:cc + 1])
                              s2_ps = p2ps.tile([128, Q], F32, tag="s2_ps",
                                                bufs=2)
                              for i in range(NI):
                                  nc.tensor.matmul(
                                      s2_ps[:],
                                      kot[i][:, cc * 128:(cc + 1) * 128],
                                      qT2[i][:],
                                      start=(i == 0), stop=(i == NI - 1),
                                      skip_group_check=True)
                              u2 = p2.tile([128, Q], BF16, tag=f"u2{cc}",
                                           name=f"u2{cc}", bufs=2)
                              nc.scalar.activation(u2[:], s2_ps[:], AF.Exp,
                                                   accum_out=Z2[:, cc:cc + 1])
                              u2s.append(u2)

                          if b == 0 and sb == 0:
                              # aw chain here: its PE ops queue after this
                              # s-block's matmuls so the AllReduce overlaps
                              nc.vector.tensor_copy(A_bf[:], A_f32[:])
                              nc.vector.scalar_tensor_tensor(
                                  scrA[:], A_f32[:], 1.0, wu2B[:],
                                  ALU.mult, ALU.mult, accum_out=aw[:])
                              nc.vector.tensor_copy(awb16[:], aw[:])
                              awT_ps = p2ps.tile([1, Q], BF16, tag="s2_ps", bufs=2)
                              nc.tensor.transpose(awT_ps[:], awb16[:],
                                                  ident[:Q, :Q])
                              nc.scalar.copy(aw_row[:], awT_ps[:])
                              awB_ps = p2ps.tile([128, Q], F32, tag="a2_ps", bufs=2)
                              nc.tensor.matmul(awB_ps[:], ones1[:],
                                               aw_row[:],
                                               start=True, stop=True)
                              nc.scalar.copy(awB[:], awB_ps[:])

                          for cc in range(CPB):
                              scr2 = p2.tile([128, Q], BF16, tag="scr2",
                                             bufs=2)
                              nc.vector.scalar_tensor_tensor(
                                  scr2[:], u2s[cc][:], 1.0, awB[:],
                                  ALU.mult, ALU.mult,
                                  accum_out=G2[:, cc:cc + 1])
                          # gates: sc = sigmoid(g1 + g2/Z + cg) / Z
                          SC = p2.tile([128, CPB], F32, tag="SC", bufs=2)
                          rz = p2.tile([128, CPB], F32, tag="rz", bufs=2)
                          nc.vector.reciprocal(rz[:], Z2[:])
                          t4 = p2.tile([128, CPB], F32, tag="t4", bufs=2)
                          nc.vector.tensor_mul(t4[:], G2[:], rz[:])
                          nc.vector.tensor_add(t4[:], t4[:], G1[:])
                          e4 = p2.tile([128, CPB], F32, tag="e4", bufs=2)
                          nc.scalar.activation(e4[:], t4[:], AF.Exp,
                                               scale=-1.0, bias=ncg[:])
                          nc.vector.tensor_scalar_add(e4[:], e4[:], 1.0)
                          nc.vector.reciprocal(e4[:], e4[:])
                          nc.vector.tensor_mul(SC[:], e4[:], rz[:])
                          # part B: transposes, then A2 matmuls + residual
                          ut2s = []
                          for cc in range(CPB):
                              ut2_ps = p2ps.tile([Q, 128], BF16, tag="ut2_ps",
                                                 bufs=2)
                              nc.tensor.transpose(ut2_ps[:], u2s[cc][:],
                                                  ident[:, :])
                              ut2 = p2.tile([Q, 128], BF16, tag="ut2", bufs=6)
                              nc.scalar.copy(ut2[:], ut2_ps[:])
                              ut2s.append(ut2)
                          for cc in range(CPB):
                              c = sb * CPB + cc
                              outt = p2.tile([128, D], F32, tag="outt",
                                             bufs=3)
                              for h in range(2):
                                  hsl = slice(h * 512, (h + 1) * 512)
                                  a2_ps = p2ps.tile([128, 512], F32,
                                                    tag="a2_ps", bufs=2)
                                  nc.tensor.matmul(a2_ps[:], ut2s[cc][:],
                                                   A_bf[:, hsl],
                                                   start=True, stop=True)
                                  nc.vector.scalar_tensor_tensor(
                                      outt[:, hsl], a2_ps[:],
                                      SC[:, cc:cc + 1],
                                      pdn[cc][:, hsl], ALU.mult, ALU.add)
                              nc.sync.dma_start(
                                  out_d[b, c * 128:(c + 1) * 128, :],
                                  outt[:])
                          pdt = nxt

            if stage == "ar":
                nc.sync.dma_start(out_d[0, 0:Q, :], A_f32[:])
                nc.gpsimd.dma_start(out_d[0, 128:256, 0:Q], awB[:])
    nc.compile()
    return nc


def _get_prog(bi_v, cgate_v):
    key = (round(bi_v, 9), round(cgate_v, 9))
    if key not in _prog_cache:
        _prog_cache[key] = _build(bi_v, cgate_v)
    return _prog_cache[key]


def kernel(raw, post_dec, mask, questions, Wk, bk, Wi, bi, Wo, bo,
           Wu1, bu1, Wu2, bu2, b1, _trace=False):
    raw = np.asarray(raw, dtype=np.float32)
    post_dec = np.asarray(post_dec, dtype=np.float32)
    questions = np.asarray(questions, dtype=np.float32)
    Wk = np.asarray(Wk, dtype=np.float32)
    Wo = np.asarray(Wo, dtype=np.float32)

    bi_v = float(np.asarray(bi).reshape(-1)[0])
    cgate_v = float(np.asarray(bu1).reshape(-1)[0]
                    + np.asarray(bu2).reshape(-1)[0]
                    + np.asarray(b1).reshape(-1)[0])
    nc = _get_prog(bi_v, cgate_v)

    inv_sqrt_d = np.float32(1.0 / np.sqrt(D))
    inv_sqrt_q = np.float32(1.0 / np.sqrt(Q))
    wkT = np.ascontiguousarray(Wk.T).astype(BF)
    woT = np.ascontiguousarray(Wo.T).astype(BF)
    qT1 = np.ascontiguousarray(questions.T * inv_sqrt_d).astype(BF)
    qT2 = np.ascontiguousarray(questions.T * inv_sqrt_q).astype(BF)
    bkT = np.ascontiguousarray(np.asarray(bk, np.float32).reshape(D, 1))
    boT = np.ascontiguousarray(np.asarray(bo, np.float32).reshape(D, 1))
    wiB = np.ascontiguousarray(
        np.broadcast_to(np.asarray(Wi, np.float32).reshape(1, D), (128, D))
    ).astype(BF)
    wu1B = np.ascontiguousarray(
        np.broadcast_to(np.asarray(Wu1, np.float32).reshape(1, D), (128, D)))
    wu2B = np.ascontiguousarray(
        np.broadcast_to(np.asarray(Wu2, np.float32).reshape(1, D), (Q, D)))

    in_maps = []
    for r in range(NCORES):
        bs = slice(r * BL, (r + 1) * BL)
        rawT = np.ascontiguousarray(
            raw[bs].transpose(0, 2, 1)).astype(BF)
        rawN = np.ascontiguousarray(raw[bs]).astype(BF)
        pdT = np.ascontiguousarray(
            post_dec[bs].transpose(0, 2, 1)).astype(BF)
        pdN = np.ascontiguousarray(post_dec[bs])
        in_maps.append({
            "rawT": rawT, "rawN": rawN, "pdT": pdT, "pdN": pdN,
            "wkT": wkT, "woT": woT, "qT1": qT1, "qT2": qT2,
            "bkT": bkT, "boT": boT, "wiB": wiB, "wu1B": wu1B, "wu2B": wu2B,
        })

    res = run_bass_kernel_spmd(nc, in_maps, core_ids=list(range(NCORES)),
                               trace=_trace)
    out = np.concatenate([res.results[r]["out"] for r in range(NCORES)],
                         axis=0)
    if _trace:
        kernel._last_result = res
    return out
